# revision 24
# baseline (speedup 1.0000x reference)
"""MoE layer (top-2, 8 experts) on 8 Trainium2 NeuronCores — eighth-H
expert sharding, bf16, software-pipelined, DMA-minimal.

Sharding: every core holds an H/8 = 512 slice of ALL 8 experts' FFNs
(16 MB SBUF, same footprint as one full expert) and processes ALL 8192
routed token-expert columns over its slice; the host sums the 8 per-core
partials. Perfectly load-balanced with no capacity caps: 8192 x 64 =
524,288 PE column-cycles per core — the exact ideal for this batch —
vs 540,416 for hot/cold expert pairing with half-H splits (-3.0%).
The compiled program depends only on the global routed-token counts
(identical on every core, SPMD-clean); all per-core variation lives in
host-packed weight slices. Measured pure-matmul floor for this exact
instruction mix is ~271 us/iter; the full kernel runs ~276 us — the
schedule is within ~5 us of the PE's own sustained rate.

Schedule: jobs are per-expert <=512-column tiles. Phase A (x@W1 + exact
gelu via ACT, 4 m-tiles of 8-deep accumulation) of job c+1 interleaves
with phase B (h@W2, 8 o-tiles of 4-deep accumulation, DVE multiplies
PSUM by the gate) of job c as [A,B,B]x4, so the PE never crosses a bulk
phase boundary. PSUM pools 4+4 banks; b2 is folded into the host
combine (exact algebra), so gelu is the only ACT table ever loaded.

DMA: everything is packed host-side into flat [128, N] job-major
layouts on a 512-column job grid; xt fetches and y flushes are nt-exact
strided single DMAs over 3D views (pad columns are never transferred,
-16% xt/y traffic), ~70 large DMAs total instead of ~180 small ones,
spread across sequencers: weights on Pool/SWDGE (later experts
quartered so the SWDGE prep rate self-throttles the weight stream below
full DMA bandwidth, keeping the xt ring fed during the cold start),
xt ring (3-deep) + g + y on SP. First expert's w1/xt land first so the
PE starts ~6 us into the dispatch. y partials ship as bf16 (hosts sums
in f32; total rel err ~4e-3 vs the 2e-2 gate).
"""

import os
from contextlib import ExitStack

import ml_dtypes
import numpy as np

import concourse.bass as bass
import concourse.tile as tile
from concourse import bacc, mybir
from concourse.bass_utils import run_bass_kernel_spmd

try:  # pragma: no cover
    import antenv.axon_hooks  # noqa: F401
except ImportError:
    os.environ.setdefault("BASS_NEVER_TRACE", "1")

BF16 = ml_dtypes.bfloat16
D, H, O, E, TOPK = 1024, 4096, 1024, 8, 2
P = 128
N_CORES = 8
HS = H // N_CORES           # 512: per-core H slice
N_D, N_HS, N_O = D // P, HS // P, O // P  # 8, 4, 8
JC = 512

_CACHE: dict[tuple, bass.Bass] = {}


def _token_tiles(C):
    if C <= 0:
        return []
    n_chunks = -(-C // JC)
    base, rem = divmod(C, n_chunks)
    tiles, t0 = [], 0
    for i in range(n_chunks):
        n = base + (1 if i < rem else 0)
        tiles.append((t0, n))
        t0 += n
    return tiles


def _jobs(counts):
    jobs = []
    for e in range(E):
        jobs += [(e, t0, nt) for (t0, nt) in _token_tiles(int(counts[e]))]
    return jobs


def _build(counts: tuple, iters: int = 1) -> bass.Bass:
    f32, bf16 = mybir.dt.float32, mybir.dt.bfloat16
    jobs = _jobs(counts)
    nj = len(jobs)
    CJ = nj * JC
    XB = N_D * JC   # 4096 columns per job block (xt and yt)
    WB = N_D * JC   # 4096 columns per expert w1/w2 block
    nc = bacc.Bacc("TRN2", target_bir_lowering=False, debug=False,
                   num_devices=N_CORES)
    xt_d = nc.dram_tensor("xt", [P, nj * N_D, JC], bf16,
                          kind="ExternalInput").ap()
    w1_d = nc.dram_tensor("w1", [P, E * WB], bf16, kind="ExternalInput").ap()
    w2_d = nc.dram_tensor("w2", [P, E * WB], bf16, kind="ExternalInput").ap()
    b1_d = nc.dram_tensor("b1", [P, E * N_HS], f32, kind="ExternalInput").ap()
    g_d = nc.dram_tensor("g", [P, CJ], bf16, kind="ExternalInput").ap()
    yt_d = nc.dram_tensor("yt", [P, nj * N_O, JC], bf16,
                          kind="ExternalOutput").ap()

    with tile.TileContext(nc) as tc, ExitStack() as ctx:
        wpool = ctx.enter_context(tc.tile_pool(name="weights", bufs=1))
        xpool = ctx.enter_context(tc.tile_pool(name="xin", bufs=1))
        hpool = ctx.enter_context(tc.tile_pool(name="hts", bufs=12))
        ppool1 = ctx.enter_context(tc.tile_pool(name="ps1", bufs=4, space="PSUM"))
        ppool2 = ctx.enter_context(tc.tile_pool(name="ps2", bufs=4, space="PSUM"))
        ypool = ctx.enter_context(tc.tile_pool(name="yout", bufs=2))

        w1_sb = wpool.tile([P, E * WB], bf16, name="w1")
        w2_sb = wpool.tile([P, E * WB], bf16, name="w2")
        b1_sb = wpool.tile([P, E * N_HS], f32, name="b1")
        g_sb = wpool.tile([P, CJ], bf16)

        RING = 3
        xt_tiles = [xpool.tile([P, N_D, JC], bf16, name=f"xt{r}")
                    for r in range(RING)]

        def fetch(ji):
            nt = jobs[ji][2]
            nc.sync.dma_start(out=xt_tiles[ji % RING][:, :, :nt],
                              in_=xt_d[:, ji * N_D:(ji + 1) * N_D, :nt])

        # Cold start: weights stream in JOB order (w1[e] just before w2[e],
        # expert by expert) so block c never waits on a load queued behind
        # weights it needs much later; g's first half lands before B(0).
        expert_order = []
        for (e, _, _) in jobs:
            if e not in expert_order:
                expert_order.append(e)
        e0 = expert_order[0]
        nt0 = jobs[0][2]
        # First job streams in d-quarters: the A-phase d-loop starts on the
        # first 2-d piece and stays just behind the arriving stream.
        for q in range(4):
            d0, d1 = q * 2, (q + 1) * 2
            nc.sync.dma_start(out=xt_tiles[0][:, d0:d1, :nt0],
                              in_=xt_d[:, d0:d1, :nt0])
            nc.gpsimd.dma_start(
                out=w1_sb[:, e0 * WB + d0 * JC:e0 * WB + d1 * JC],
                in_=w1_d[:, e0 * WB + d0 * JC:e0 * WB + d1 * JC])
        nc.gpsimd.dma_start(out=b1_sb[:], in_=b1_d[:])
        fetch(1)
        nc.gpsimd.dma_start(out=w2_sb[:, e0 * WB:(e0 + 1) * WB],
                            in_=w2_d[:, e0 * WB:(e0 + 1) * WB])
        fetch(2)
        gq = CJ // 4
        nc.sync.dma_start(out=g_sb[:, :gq], in_=g_d[:, :gq])
        for i, e in enumerate(expert_order[1:], 1):
            # Quarter the later experts' loads: the SWDGE prep rate then
            # self-throttles the weight stream to ~2/3 of DMA bandwidth,
            # so the xt ring's fetches never starve behind it.
            pieces = 1 if i == 1 else 4
            for w_dram, w_tile in ((w1_d, w1_sb), (w2_d, w2_sb)):
                for q in range(pieces):
                    sz = WB // pieces
                    off = e * WB + q * sz
                    nc.gpsimd.dma_start(out=w_tile[:, off:off + sz],
                                        in_=w_dram[:, off:off + sz])
            if 1 <= i <= 3:
                nc.sync.dma_start(out=g_sb[:, i * gq:(i + 1) * gq],
                                  in_=g_d[:, i * gq:(i + 1) * gq])

        gelu = mybir.ActivationFunctionType.Gelu

        loop_ctx = ExitStack()
        if iters > 1:
            loop_ctx.enter_context(tc.For_i(0, iters, 1))
        ctx.enter_context(loop_ctx)

        def a_group(ji, m):
            (e, t0, nt) = jobs[ji]
            ps = ppool1.tile([P, JC], f32, tag="ps1")
            xt_t = xt_tiles[ji % RING]
            for d in range(N_D):
                nc.tensor.matmul(ps[:, :nt],
                                 lhsT=w1_sb[:, e * WB + d * JC + m * P:
                                            e * WB + d * JC + m * P + P],
                                 rhs=xt_t[:, d, :nt],
                                 start=(d == 0), stop=(d == N_D - 1))
            ht = hpool.tile([P, JC], bf16, tag="ht")
            nc.scalar.activation(ht[:, :nt], ps[:, :nt], gelu,
                                 bias=b1_sb[:, e * N_HS + m:e * N_HS + m + 1])
            return ht

        def b_group(ji, o, hts, ym):
            (e, t0, nt) = jobs[ji]
            ps2 = ppool2.tile([P, JC], f32, tag="ps2")
            for h in range(N_HS):
                nc.tensor.matmul(ps2[:, :nt],
                                 lhsT=w2_sb[:, e * WB + h * O + o * P:
                                            e * WB + h * O + o * P + P],
                                 rhs=hts[h][:, :nt],
                                 start=(h == 0), stop=(h == N_HS - 1))
            nc.vector.tensor_mul(ym[:, o % 4, :nt],
                                 ps2[:, :nt], g_sb[:, ji * JC:ji * JC + nt])

        def b_block(c, hts_cur, interleave):
            last = interleave is None
            ym = None
            for o in range(N_O):
                if o % 4 == 0:
                    ym = ypool.tile([P, 4, JC], bf16, tag="ym")
                if interleave is not None and o % 2 == 0:
                    interleave(o // 2)
                b_group(c, o, hts_cur, ym)
                nt = jobs[c][2]
                if last:
                    nc.sync.dma_start(out=yt_d[:, c * N_O + o, :nt],
                                      in_=ym[:, o % 4, :nt])
                elif o % 4 == 3:
                    ro = c * N_O + (o // 4) * 4
                    nc.sync.dma_start(out=yt_d[:, ro:ro + 4, :nt],
                                      in_=ym[:, :, :nt])

        hts_cur = [a_group(0, m) for m in range(N_HS)]
        hts_next = []
        for c in range(nj):
            nxt = c + RING
            if nxt < nj:
                fetch(nxt)
            if iters > 1:
                for w_ in range(RING):
                    if w_ + RING * ((nj - 1 - w_) // RING) == c:
                        fetch(w_)
            if c + 1 < nj:
                hts_next = []

                def interleave(i, _c=c):
                    hts_next.append(a_group(_c + 1, i))

                b_block(c, hts_cur, interleave)
                hts_cur = hts_next
            else:
                b_block(c, hts_cur, None)
    nc.compile()
    return nc


def _prepare(x, Wg, W1, b1, W2, b2):
    """Host gating + per-core packed input maps (eighth-H sharding)."""
    x = np.asarray(x)
    B, S, Dx = x.shape
    assert Dx == D and Wg.shape == (D, E)
    T = B * S
    xf = np.ascontiguousarray(x.reshape(T, D), dtype=np.float32)
    logits = xf.astype(np.float64) @ np.asarray(Wg, np.float64)
    top_i = np.argpartition(-logits, TOPK - 1, axis=1)[:, :TOPK]
    lv = np.take_along_axis(logits, top_i, axis=1)
    lv -= lv.max(axis=1, keepdims=True)
    ex_ = np.exp(lv)
    w = ex_ / ex_.sum(axis=1, keepdims=True)

    flat_e = top_i.reshape(-1)
    flat_w = w.reshape(-1)
    counts = tuple(int(c) for c in np.bincount(flat_e, minlength=E))

    jobs = _jobs(counts)
    nj = len(jobs)
    CJ = nj * JC
    # per-expert cap-index -> padded job-grid column
    colmap = {e: np.empty(counts[e], np.int64) for e in range(E)}
    for ji, (e, t0, nt) in enumerate(jobs):
        colmap[e][t0:t0 + nt] = ji * JC + np.arange(nt)

    xt_bf = np.ascontiguousarray(xf.T).astype(BF16)  # [D, T]
    W1b_ = np.asarray(W1).astype(BF16)
    W2b_ = np.asarray(W2).astype(BF16)
    b1f = np.asarray(b1, np.float32)

    xt_p = np.zeros((D, CJ), dtype=BF16)
    g_p = np.zeros((CJ,), dtype=np.float32)
    glob = np.empty(2 * T, dtype=np.int64)
    for e in range(E):
        sel = np.nonzero(flat_e == e)[0]
        cols = colmap[e]
        xt_p[:, cols] = xt_bf[:, sel >> 1]
        g_p[cols] = flat_w[sel]
        glob[sel] = cols
    xt_pk = np.ascontiguousarray(
        xt_p.reshape(N_D, P, nj, JC).transpose(1, 2, 0, 3)
        .reshape(P, nj * N_D, JC))
    g_bc = np.ascontiguousarray(np.broadcast_to(g_p, (P, CJ)).astype(BF16))

    in_maps = []
    for c in range(N_CORES):
        sl = slice(c * HS, (c + 1) * HS)
        w1c = np.concatenate([
            W1b_[e][:, sl].reshape(N_D, P, HS).transpose(1, 0, 2)
            .reshape(P, N_D * HS) for e in range(E)], axis=1)
        w2c = np.concatenate([
            W2b_[e][sl, :].reshape(N_HS, P, O).transpose(1, 0, 2)
            .reshape(P, N_HS * O) for e in range(E)], axis=1)
        b1c = np.concatenate([
            b1f[e][sl].reshape(N_HS, P).T for e in range(E)], axis=1)
        in_maps.append({
            "xt": xt_pk,
            "w1": np.ascontiguousarray(w1c),
            "w2": np.ascontiguousarray(w2c),
            "b1": np.ascontiguousarray(b1c),
            "g": g_bc,
        })
    b2f = np.asarray(b2, np.float32)
    corr = (w[:, 0:1] * b2f[top_i[:, 0]]
            + w[:, 1:2] * b2f[top_i[:, 1]]).astype(np.float32)
    return in_maps, (glob, corr), counts, B, S


def _get_nc(counts, iters: int = 1) -> bass.Bass:
    key = (counts, iters)
    nc = _CACHE.get(key)
    if nc is None:
        nc = _CACHE[key] = _build(counts, iters)
    return nc


def _combine(results, glob_corr, counts, B, S):
    glob, corr = glob_corr
    nj = len(_jobs(counts))
    CJ = nj * JC
    # Sum the 8 per-core partials first (pad columns hold junk that can
    # overflow in f32 -- they are never indexed by glob; mask the warning).
    with np.errstate(over="ignore", invalid="ignore"):
        Ysum = None
        for r in results:
            Yc = np.asarray(r["yt"]).reshape(P, nj, N_O, JC) \
                .transpose(1, 3, 2, 0).reshape(CJ, O).astype(np.float32)
            Ysum = Yc if Ysum is None else Ysum + Yc
        out = Ysum[glob[0::2]] + Ysum[glob[1::2]] + corr
    return out.reshape(B, S, O).astype(np.float32, copy=False)


def _spot_check(out, x, Wg, W1, b1, W2, b2):
    """Exact host recompute of a few tokens; guards against the rare
    corrupted first dispatch seen on freshly attached axon devices."""
    import math

    erf = np.vectorize(math.erf)
    B, S, _ = out.shape
    T = B * S
    toks = [0, T // 3, (2 * T) // 3, T - 1]
    xf = np.asarray(x, np.float64).reshape(T, D)[toks]
    logits = xf @ np.asarray(Wg, np.float64)
    top2 = np.argsort(-logits, axis=1)[:, :TOPK]
    for i, t in enumerate(toks):
        lv = logits[i, top2[i]]
        wv = np.exp(lv - lv.max())
        wv /= wv.sum()
        y = np.zeros(O)
        for k in range(TOPK):
            e = top2[i, k]
            h = xf[i] @ np.asarray(W1[e], np.float64) + np.asarray(b1[e], np.float64)
            h = 0.5 * h * (1.0 + erf(h / np.sqrt(2.0)))
            y += wv[k] * (h @ np.asarray(W2[e], np.float64)
                          + np.asarray(b2[e], np.float64))
        got = out.reshape(T, O)[t]
        if np.abs(got - y).max() > 0.12 * (np.abs(y).max() + 1.0):
            return False
    return True


def kernel(x, Wg, W1, b1, W2, b2):
    in_maps, glob, counts, B, S = _prepare(x, Wg, W1, b1, W2, b2)
    nc = _get_nc(counts)
    for _attempt in range(2):
        res = run_bass_kernel_spmd(nc, in_maps, core_ids=list(range(N_CORES)))
        out = _combine(res.results, glob, counts, B, S)
        if _spot_check(out, x, Wg, W1, b1, W2, b2):
            break
    return out
